# revision 64
# baseline (speedup 1.0000x reference)
"""Trainium2 Bass kernel for a 3x3 stride-1 pad-1 Conv2d.

Problem: x (16, 64, 112, 112) f32, weights (128, 64, 9) f32
         -> out (16, 128, 112, 112) f32  (no bias)

Strategy (8 NeuronCores, data parallel over batch):
  - Each core gets 2 images. Image 0 lives in SBUF partitions 0-63
    (64 input channels), image 1 in partitions 64-127, both stored as a
    zero-padded (114, 114) plane per channel. The zero padding is
    materialized on the host (xp input), so every input DMA is a fully
    contiguous fat-descriptor transfer straight into the padded plane.
  - Inputs, weights and outputs travel as bf16 (host-side cast); PSUM
    accumulation stays fp32. The conv's 576-term dot products keep the
    relative error ~3e-3, well inside the 2e-2 gate, while halving HBM
    traffic and enabling the PE's fast-weight-load path (FWL is fp32-off),
    which the fp32r version was bottlenecked on (504 x ~107ns LDWEIGHTS).
  - Conv = 9 shift-and-matmul taps accumulated in PSUM: for each tap
    (dy, dx), matmul with lhsT = w[tap] (64 x 128: in-ch x out-ch) and
    rhs = shifted x window (64 x 448: in-ch x 4 output rows).
  - The two images' matmuls use disjoint PE row groups (rows 0-63 vs
    64-127 via tile_position), so they execute concurrently -> together
    they fill the whole 128x128 array despite the 64-deep contraction.
    PE floor: 28 blocks x 9 taps x 448 cols = 112,896 cycles ~= 47us.
  - The PE clock-gate (HAM) boots at half rate and only reaches 2.4 GHz
    after ~4.5us of sustained matmul activity, so a stream of dummy
    warmup matmuls runs while the first input band + weights are still
    in flight, sized to hand over to the real stream right as its data
    lands (~11us in). Input bands ride the sync ring as one DMA per
    band covering both images (FIFO order drains the 6-row head band
    first); weights ride the scalar ring in parallel.
  - Outputs are staged per 8-row band in SBUF; PSUM -> SBUF copies
    (with the fp32->bf16 cast) run on VectorE, stores on the scalar
    HWDGE (image 0) and gpsimd SWDGE (image 1) descriptor-gen paths.
    The final band stores per 4-row block on the engines that are idle
    at the end, so the drain tail after the last matmul is short.
"""

import numpy as np

import concourse.bacc as bacc
import concourse.mybir as mybir
import concourse.tile as tile
from concourse.bass_utils import run_bass_kernel_spmd

N_CORES = 8
B, C, H, W = 16, 64, 112, 112
O = 128
BPC = B // N_CORES          # images per core
HP = H + 2                  # padded rows per image plane
WP = W + 2                  # padded cols
NTAPS = 9
RPB = 4                     # output rows per block (free dim = 4*112 = 448)
NBLOCKS = H // RPB          # 28
BAND = 16                   # output rows per output band
NBANDS = H // BAND          # 7

F32 = mybir.dt.float32
BF16 = mybir.dt.bfloat16

try:
    import ml_dtypes
    NP_BF16 = ml_dtypes.bfloat16
except ImportError:  # pragma: no cover
    NP_BF16 = mybir.dt.np(mybir.dt.bfloat16)

# input bands over padded rows: (first padded row, nrows). The head band
# covers only block 0 so the PE starts ASAP; the bands drain in FIFO
# order on the sync ring.
_IN_BANDS = [(0, 5), (5, 9), (14, 16), (30, 16), (46, 16), (62, 16),
             (78, 16), (94, 16), (110, 4)]


def _conv_body(tc, out_ap, xp_ap, w_ap):
    nc = tc.nc
    from contextlib import ExitStack

    with ExitStack() as ctx:
        xpool = ctx.enter_context(tc.tile_pool(name="xb", bufs=1))
        wpool = ctx.enter_context(tc.tile_pool(name="wt", bufs=1))
        pspool = ctx.enter_context(tc.tile_pool(name="ps", bufs=4, space="PSUM"))
        opool = ctx.enter_context(tc.tile_pool(name="ob", bufs=4))

        # x planes: partitions [64*im, 64*im+64) hold image im, padded.
        xb = xpool.tile([128, HP, WP], BF16)
        # weights: wt[p, t, m] = w[m, p % 64, t] (taps replicated per half)
        wt = wpool.tile([128, NTAPS, O], BF16)

        # HAM warmup: the PE clock-gate defaults to 4/8 (1.2 GHz) and takes
        # ~4.5us of sustained full-array matmul activity to reach 8/8, a
        # clock that starts ticking at the first matmul no matter what. So
        # start dummy matmuls over a zeroed scratch tile as soon as the
        # engines boot, sized to hand over to the real stream the moment
        # its first input band + weights have landed.
        scr = wpool.tile([128, 128], BF16, name="wu_scr", tag="scr")
        wups = pspool.tile([128, 128], F32, name="wu_ps", tag="ps0")
        nc.gpsimd.memset(scr[:], 0.0)
        for _ in range(34):
            nc.tensor.matmul(
                wups[:], scr[:], scr[:],
                start=True, stop=True, tile_position=(0, 0),
            )

        # weights ride the scalar engine's HWDGE ring so their descriptor
        # generation runs in parallel with the input bands' on the sync
        # ring. Bands are unchained: the sync ring's HWDGE processes its
        # dma_starts in FIFO order, so the bands drain head-band-first at
        # full bandwidth without per-link semaphore gaps.
        nc.scalar.dma_start(out=wt[:], in_=w_ap[:])
        for bi, (r0, n) in enumerate(_IN_BANDS):
            # one DMA covers both images: linearized (im, ch) order on the
            # DRAM side matches partition-major order on the SBUF side, so
            # each band costs a single descriptor-gen pass + one semaphore.
            nc.sync.dma_start(
                out=xb[:, r0:r0 + n, :],
                in_=xp_ap[:, :, r0:r0 + n, :],
            )

        ob_tiles = {}
        for p in range(NBLOCKS):
            r = RPB * p
            band = r // BAND
            boff = r - band * BAND
            if boff == 0:
                for im in range(BPC):
                    ob_tiles[im] = opool.tile(
                        [128, BAND, W], BF16, name=f"ob{im}_{band}", tag=f"ob{im}"
                    )
            ps = [
                pspool.tile([128, RPB, W], F32, tag=f"ps{im}", name=f"ps{im}_{p}")
                for im in range(BPC)
            ]
            for t in range(NTAPS):
                i, j = divmod(t, 3)
                first, last = t == 0, t == NTAPS - 1
                for im in range(BPC):
                    p0 = 64 * im
                    nc.tensor.matmul(
                        ps[im][:],
                        wt[p0:p0 + 64, t, :],
                        xb[p0:p0 + 64, r + i:r + i + RPB, j:j + W],
                        start=first,
                        stop=last,
                        tile_position=(p0, 0),
                    )
            if p == NBLOCKS - 1:
                # final block: run the two casts in parallel on scalar +
                # vector to shorten the drain tail (the scalar ACT table
                # loads at boot, hidden under the DMA-ring bring-up).
                nc.scalar.copy(ob_tiles[0][:, boff:boff + RPB, :], ps[0][:])
                nc.vector.tensor_copy(ob_tiles[1][:, boff:boff + RPB, :], ps[1][:])
            else:
                for im in range(BPC):
                    nc.vector.tensor_copy(
                        ob_tiles[im][:, boff:boff + RPB, :], ps[im][:])
            last_band = band == NBANDS - 1
            if last_band:
                # final band: store each 4-row block as soon as its cast
                # lands, so descriptor generation + most of the transfer
                # overlap the last block's matmuls instead of trailing them.
                # The very last pair rides the two engines that are idle at
                # the end (sync / scalar) so their descriptor generation
                # runs in parallel right after the casts.
                st0, st1 = (nc.sync, nc.scalar) if p == NBLOCKS - 1 \
                    else (nc.scalar, nc.sync)
                st0.dma_start(
                    out=out_ap[0, :, r:r + RPB, :],
                    in_=ob_tiles[0][:, boff:boff + RPB, :],
                )
                st1.dma_start(
                    out=out_ap[1, :, r:r + RPB, :],
                    in_=ob_tiles[1][:, boff:boff + RPB, :],
                )
            elif boff + RPB == BAND:
                # one store per image per band, on separate descriptor-gen
                # paths (scalar HWDGE / gpsimd SWDGE): DIRECT2D descriptor
                # generation costs ~0.6us of sequencer time per store, so
                # keeping each engine's FIFO to one store per band avoids a
                # backlog that would drain serially after the last matmul.
                nc.scalar.dma_start(
                    out=out_ap[0, :, band * BAND:(band + 1) * BAND, :],
                    in_=ob_tiles[0][:],
                )
                nc.sync.dma_start(
                    out=out_ap[1, :, band * BAND:(band + 1) * BAND, :],
                    in_=ob_tiles[1][:],
                )


def build_program():
    nc = bacc.Bacc("TRN2", target_bir_lowering=False, num_devices=N_CORES)
    x_t = nc.dram_tensor("xp", [BPC, C, HP, WP], BF16, kind="ExternalInput")
    w_t = nc.dram_tensor("wT", [128, NTAPS, O], BF16, kind="ExternalInput")
    o_t = nc.dram_tensor("out", [BPC, O, H, W], BF16, kind="ExternalOutput")
    with tile.TileContext(nc) as tc:
        _conv_body(tc, o_t.ap(), x_t.ap(), w_t.ap())
    nc.compile()
    return nc


def pack_weights(weights: np.ndarray) -> np.ndarray:
    # (O, C, 9) -> (128, 9, O) with wT[p, t, m] = weights[m, p % 64, t]
    wT = np.ascontiguousarray(np.transpose(weights, (1, 2, 0)))  # (C, 9, O)
    return np.ascontiguousarray(
        np.concatenate([wT, wT], axis=0).astype(NP_BF16))


def pad_input(x: np.ndarray) -> np.ndarray:
    # (B, C, H, W) -> (B, C, H+2, W+2) zero-padded, bf16
    xp = np.zeros((x.shape[0], x.shape[1], HP, WP), NP_BF16)
    xp[:, :, 1:1 + H, 1:1 + W] = x.astype(NP_BF16)
    return xp


def run(x: np.ndarray, weights: np.ndarray, **spmd_kwargs):
    x = np.ascontiguousarray(x, dtype=np.float32)
    w = np.ascontiguousarray(weights, dtype=np.float32)
    wT = pack_weights(w)
    xp = pad_input(x)
    nc = build_program()
    in_maps = [
        {"xp": xp[BPC * i:BPC * (i + 1)], "wT": wT} for i in range(N_CORES)
    ]
    res = run_bass_kernel_spmd(nc, in_maps, list(range(N_CORES)), **spmd_kwargs)
    outs = [
        np.asarray(res.results[i]["out"]).reshape(BPC, O, H, W)
        for i in range(N_CORES)
    ]
    return np.concatenate(outs, axis=0).astype(np.float32), res


def kernel(x: np.ndarray, weights: np.ndarray) -> np.ndarray:
    out, _ = run(x, weights)
    return out


# revision 65
# speedup vs baseline: 1.1738x; 1.1738x over previous
"""Trainium2 Bass kernel for a 3x3 stride-1 pad-1 Conv2d.

Problem: x (16, 64, 112, 112) f32, weights (128, 64, 9) f32
         -> out (16, 128, 112, 112) f32  (no bias)

Strategy (8 NeuronCores, data parallel over batch):
  - Each core gets 2 images. Image 0 lives in SBUF partitions 0-63
    (64 input channels), image 1 in partitions 64-127, both stored as a
    zero-padded (114, 114) plane per channel. The zero padding is
    materialized on the host (xp input), so every input DMA is a fully
    contiguous fat-descriptor transfer straight into the padded plane.
  - Inputs, weights and outputs travel as bf16 (host-side cast); PSUM
    accumulation stays fp32. The conv's 576-term dot products keep the
    relative error ~3e-3, well inside the 2e-2 gate, while halving HBM
    traffic and enabling the PE's fast-weight-load path (FWL is fp32-off),
    which the fp32r version was bottlenecked on (504 x ~107ns LDWEIGHTS).
  - Conv = 9 shift-and-matmul taps accumulated in PSUM: for each tap
    (dy, dx), matmul with lhsT = w[tap] (64 x 128: in-ch x out-ch) and
    rhs = shifted x window (64 x 448: in-ch x 4 output rows).
  - The two images' matmuls use disjoint PE row groups (rows 0-63 vs
    64-127 via tile_position), so they execute concurrently -> together
    they fill the whole 128x128 array despite the 64-deep contraction.
    PE floor: 28 blocks x 9 taps x 448 cols = 112,896 cycles ~= 47us.
  - The PE clock-gate (HAM) boots at half rate and only reaches 2.4 GHz
    after ~4.5us of sustained matmul activity, so a stream of dummy
    warmup matmuls runs while the first input band + weights are still
    in flight, sized to hand over to the real stream right as its data
    lands (~11us in). Input bands ride the sync ring as one DMA per
    band covering both images (FIFO order drains the 6-row head band
    first); weights ride the scalar ring in parallel.
  - Outputs are staged per 8-row band in SBUF; PSUM -> SBUF copies
    (with the fp32->bf16 cast) run on VectorE, stores on the scalar
    HWDGE (image 0) and gpsimd SWDGE (image 1) descriptor-gen paths.
    The final band stores per 4-row block on the engines that are idle
    at the end, so the drain tail after the last matmul is short.
"""

import numpy as np

import concourse.bacc as bacc
import concourse.mybir as mybir
import concourse.tile as tile
from concourse.bass_utils import run_bass_kernel_spmd

N_CORES = 8
B, C, H, W = 16, 64, 112, 112
O = 128
BPC = B // N_CORES          # images per core
HP = H + 2                  # padded rows per image plane
WP = W + 2                  # padded cols
NTAPS = 9
RPB = 4                     # output rows per block (free dim = 4*112 = 448)
NBLOCKS = H // RPB          # 28
BAND = 8                    # output rows per output band
NBANDS = H // BAND          # 14

F32 = mybir.dt.float32
BF16 = mybir.dt.bfloat16

try:
    import ml_dtypes
    NP_BF16 = ml_dtypes.bfloat16
except ImportError:  # pragma: no cover
    NP_BF16 = mybir.dt.np(mybir.dt.bfloat16)

# input bands over padded rows: (first padded row, nrows). The head band
# covers only block 0 so the PE starts ASAP; the bands drain in FIFO
# order on the sync ring.
_IN_BANDS = [(0, 5), (5, 9), (14, 16), (30, 16), (46, 16), (62, 16),
             (78, 16), (94, 16), (110, 4)]


def _conv_body(tc, out_ap, xp_ap, w_ap):
    nc = tc.nc
    from contextlib import ExitStack

    with ExitStack() as ctx:
        xpool = ctx.enter_context(tc.tile_pool(name="xb", bufs=1))
        wpool = ctx.enter_context(tc.tile_pool(name="wt", bufs=1))
        pspool = ctx.enter_context(tc.tile_pool(name="ps", bufs=4, space="PSUM"))
        opool = ctx.enter_context(tc.tile_pool(name="ob", bufs=4))

        # x planes: partitions [64*im, 64*im+64) hold image im, padded.
        xb = xpool.tile([128, HP, WP], BF16)
        # weights: wt[p, t, m] = w[m, p % 64, t] (taps replicated per half)
        wt = wpool.tile([128, NTAPS, O], BF16)

        # HAM warmup: the PE clock-gate defaults to 4/8 (1.2 GHz) and takes
        # ~4.5us of sustained full-array matmul activity to reach 8/8, a
        # clock that starts ticking at the first matmul no matter what. So
        # start dummy matmuls over a zeroed scratch tile as soon as the
        # engines boot, sized to hand over to the real stream the moment
        # its first input band + weights have landed.
        scr = wpool.tile([128, 128], BF16, name="wu_scr", tag="scr")
        wups = pspool.tile([128, 128], F32, name="wu_ps", tag="ps0")
        nc.gpsimd.memset(scr[:], 0.0)
        for _ in range(34):
            nc.tensor.matmul(
                wups[:], scr[:], scr[:],
                start=True, stop=True, tile_position=(0, 0),
            )

        # weights ride the scalar engine's HWDGE ring so their descriptor
        # generation runs in parallel with the input bands' on the sync
        # ring. Bands are unchained: the sync ring's HWDGE processes its
        # dma_starts in FIFO order, so the bands drain head-band-first at
        # full bandwidth without per-link semaphore gaps.
        nc.scalar.dma_start(out=wt[:], in_=w_ap[:])
        for bi, (r0, n) in enumerate(_IN_BANDS):
            # one DMA covers both images: linearized (im, ch) order on the
            # DRAM side matches partition-major order on the SBUF side, so
            # each band costs a single descriptor-gen pass + one semaphore.
            nc.sync.dma_start(
                out=xb[:, r0:r0 + n, :],
                in_=xp_ap[:, :, r0:r0 + n, :],
            )

        ob_tiles = {}
        for p in range(NBLOCKS):
            r = RPB * p
            band = r // BAND
            boff = r - band * BAND
            if boff == 0:
                for im in range(BPC):
                    ob_tiles[im] = opool.tile(
                        [128, BAND, W], BF16, name=f"ob{im}_{band}", tag=f"ob{im}"
                    )
            ps = [
                pspool.tile([128, RPB, W], F32, tag=f"ps{im}", name=f"ps{im}_{p}")
                for im in range(BPC)
            ]
            for t in range(NTAPS):
                i, j = divmod(t, 3)
                first, last = t == 0, t == NTAPS - 1
                for im in range(BPC):
                    p0 = 64 * im
                    nc.tensor.matmul(
                        ps[im][:],
                        wt[p0:p0 + 64, t, :],
                        xb[p0:p0 + 64, r + i:r + i + RPB, j:j + W],
                        start=first,
                        stop=last,
                        tile_position=(p0, 0),
                    )
            if p == NBLOCKS - 1:
                # final block: run the two casts in parallel on scalar +
                # vector to shorten the drain tail (the scalar ACT table
                # loads at boot, hidden under the DMA-ring bring-up).
                nc.scalar.copy(ob_tiles[0][:, boff:boff + RPB, :], ps[0][:])
                nc.vector.tensor_copy(ob_tiles[1][:, boff:boff + RPB, :], ps[1][:])
            else:
                for im in range(BPC):
                    nc.vector.tensor_copy(
                        ob_tiles[im][:, boff:boff + RPB, :], ps[im][:])
            last_band = band == NBANDS - 1
            if last_band:
                # final band: store each 4-row block as soon as its cast
                # lands, so descriptor generation + most of the transfer
                # overlap the last block's matmuls instead of trailing them.
                # The very last pair rides the two engines that are idle at
                # the end (sync / scalar) so their descriptor generation
                # runs in parallel right after the casts.
                st0, st1 = (nc.sync, nc.scalar) if p == NBLOCKS - 1 \
                    else (nc.scalar, nc.sync)
                st0.dma_start(
                    out=out_ap[0, :, r:r + RPB, :],
                    in_=ob_tiles[0][:, boff:boff + RPB, :],
                )
                st1.dma_start(
                    out=out_ap[1, :, r:r + RPB, :],
                    in_=ob_tiles[1][:, boff:boff + RPB, :],
                )
            elif boff + RPB == BAND:
                # one store per image per band, on separate descriptor-gen
                # paths (scalar HWDGE / gpsimd SWDGE): DIRECT2D descriptor
                # generation costs ~0.6us of sequencer time per store, so
                # keeping each engine's FIFO to one store per band avoids a
                # backlog that would drain serially after the last matmul.
                nc.scalar.dma_start(
                    out=out_ap[0, :, band * BAND:(band + 1) * BAND, :],
                    in_=ob_tiles[0][:],
                )
                nc.sync.dma_start(
                    out=out_ap[1, :, band * BAND:(band + 1) * BAND, :],
                    in_=ob_tiles[1][:],
                )


def build_program():
    nc = bacc.Bacc("TRN2", target_bir_lowering=False, num_devices=N_CORES)
    x_t = nc.dram_tensor("xp", [BPC, C, HP, WP], BF16, kind="ExternalInput")
    w_t = nc.dram_tensor("wT", [128, NTAPS, O], BF16, kind="ExternalInput")
    o_t = nc.dram_tensor("out", [BPC, O, H, W], BF16, kind="ExternalOutput")
    with tile.TileContext(nc) as tc:
        _conv_body(tc, o_t.ap(), x_t.ap(), w_t.ap())
    nc.compile()
    return nc


def pack_weights(weights: np.ndarray) -> np.ndarray:
    # (O, C, 9) -> (128, 9, O) with wT[p, t, m] = weights[m, p % 64, t]
    wT = np.ascontiguousarray(np.transpose(weights, (1, 2, 0)))  # (C, 9, O)
    return np.ascontiguousarray(
        np.concatenate([wT, wT], axis=0).astype(NP_BF16))


def pad_input(x: np.ndarray) -> np.ndarray:
    # (B, C, H, W) -> (B, C, H+2, W+2) zero-padded, bf16
    xp = np.zeros((x.shape[0], x.shape[1], HP, WP), NP_BF16)
    xp[:, :, 1:1 + H, 1:1 + W] = x.astype(NP_BF16)
    return xp


def run(x: np.ndarray, weights: np.ndarray, **spmd_kwargs):
    x = np.ascontiguousarray(x, dtype=np.float32)
    w = np.ascontiguousarray(weights, dtype=np.float32)
    wT = pack_weights(w)
    xp = pad_input(x)
    nc = build_program()
    in_maps = [
        {"xp": xp[BPC * i:BPC * (i + 1)], "wT": wT} for i in range(N_CORES)
    ]
    res = run_bass_kernel_spmd(nc, in_maps, list(range(N_CORES)), **spmd_kwargs)
    outs = [
        np.asarray(res.results[i]["out"]).reshape(BPC, O, H, W)
        for i in range(N_CORES)
    ]
    return np.concatenate(outs, axis=0).astype(np.float32), res


def kernel(x: np.ndarray, weights: np.ndarray) -> np.ndarray:
    out, _ = run(x, weights)
    return out
